# revision 31
# baseline (speedup 1.0000x reference)
"""Causal multi-head attention block on 8 trn2 NeuronCores.

Problem (hardcoded): x [4, 2048, 1024] fp32, W_attn [1024, 3072], W_proj
[1024, 1024]; H=16 heads, D=64; scores scaled by 1/sqrt(1024); causal
softmax; y @ W_proj.

Sharding: core c -> (batch b = c//2, head-group hg = c%2 of 8 heads).
Each core computes q,k,v for its batch + head-group, causal attention,
and a partial projection out_partial = y_slice @ W_proj[rows of its
head-group].  Host sums the two partials per batch.

Device-side layout trick: everything is computed transposed (d on
partitions, tokens on the free axis) so no on-device transposes are
needed:
  qT/kT = W_slice.T @ xT          (xT passed pre-transposed from host)
  sT[j,i] = k_j . q_i             (lhsT = kT tile, rhs = qT range)
  ET = exp(sT/32)                 (no max-subtraction: |s/32| < ~1.5 by
                                   construction of the input distribution)
  yT_un[d,i], Z[i] = v_aug.T @ ET (v_aug has an all-ones 65th column, so
                                   row 64 of the product is the softmax
                                   denominator -- free on the PE)
  out = (yT_un/Z).T @ W_proj_slice
"""

import os
from contextlib import ExitStack

import numpy as np
import ml_dtypes

import concourse.bass as bass
import concourse.mybir as mybir
from concourse import bacc, tile
from concourse.bass_utils import run_bass_kernel_spmd

B, L, C, H, D = 4, 2048, 1024, 16, 64
P = 128
NCORES = 8
NH = 8          # heads per core
NPAIR = 4       # head pairs per core
CK = C // P     # 8 contraction k-tiles over C
NCH = 4         # 512-token chunks per batch
NR = 4          # query i-ranges of 512
NJT = 16        # key j-tiles of 128
BF16 = mybir.dt.bfloat16
F32 = mybir.dt.float32

_COMPILED = None


def _build_program(reps=1):
    nc = bacc.Bacc("TRN2", target_bir_lowering=False, debug=False,
                   num_devices=NCORES)
    xT_d = nc.dram_tensor("xt", [C, L], BF16, kind="ExternalInput")
    wq_d = nc.dram_tensor("wq", [C, 512], BF16, kind="ExternalInput")
    wk_d = nc.dram_tensor("wk", [C, 512], BF16, kind="ExternalInput")
    wv_d = nc.dram_tensor("wv", [C, 512], BF16, kind="ExternalInput")
    wp_d = nc.dram_tensor("wp", [512, C], BF16, kind="ExternalInput")
    mk_d = nc.dram_tensor("mk", [P, 2048], BF16, kind="ExternalInput")
    sel_d = nc.dram_tensor("sel", [8, 512], BF16, kind="ExternalInput")
    out_d = nc.dram_tensor("out", [L, C], F32, kind="ExternalOutput")

    with tile.TileContext(nc) as tc, ExitStack() as ctx:
        const = ctx.enter_context(tc.tile_pool(name="const", bufs=1))
        etp = ctx.enter_context(tc.tile_pool(name="et", bufs=4))
        ysp = ctx.enter_context(tc.tile_pool(name="ys", bufs=16))
        zgp = ctx.enter_context(tc.tile_pool(name="zg", bufs=3))
        z0p = ctx.enter_context(tc.tile_pool(name="z0", bufs=3))
        zbp = ctx.enter_context(tc.tile_pool(name="zb", bufs=2))
        ytp = ctx.enter_context(tc.tile_pool(name="ytmp", bufs=4))
        op = ctx.enter_context(tc.tile_pool(name="ob", bufs=2))
        ps = ctx.enter_context(
            tc.tile_pool(name="ps", bufs=2, space=bass.MemorySpace.PSUM))
        py = ctx.enter_context(
            tc.tile_pool(name="py", bufs=2, space=bass.MemorySpace.PSUM))
        pp = ctx.enter_context(
            tc.tile_pool(name="pp", bufs=2, space=bass.MemorySpace.PSUM))

        xT = const.tile([P, CK, L], BF16)
        wq = const.tile([P, CK, 512], BF16)
        wk = const.tile([P, CK, 512], BF16)
        wv = const.tile([P, CK, 512], BF16)
        wp = const.tile([P, NPAIR, C], BF16)
        mk = const.tile([P, 2048], BF16)
        qT = const.tile([P, NPAIR, L], BF16)
        kT = const.tile([P, NPAIR, L], BF16)
        vsb = const.tile([P, NH, NJT, 65], BF16)
        yT = const.tile([P, NPAIR, L], BF16)
        sel8 = const.tile([8, 8, 64], BF16)

        # DMA in consumption order: first QKV chunk needs wq + xT chunk 0
        # first, so the PE can start ~14us earlier than a bulk load.
        xT_v = xT_d.ap().rearrange("(k p) n -> p k n", p=P)
        nc.sync.dma_start(wq[:], wq_d.ap().rearrange("(k p) n -> p k n", p=P))
        for k in range(CK):
            nc.sync.dma_start(xT[:, k, 0:512], xT_v[:, k, 0:512])
        nc.sync.dma_start(wk[:], wk_d.ap().rearrange("(k p) n -> p k n", p=P))
        nc.sync.dma_start(wv[:], wv_d.ap().rearrange("(k p) n -> p k n", p=P))
        nc.sync.dma_start(mk[:], mk_d.ap())
        for ch in range(1, NCH):
            cs = slice(ch * 512, (ch + 1) * 512)
            for k in range(CK):
                nc.sync.dma_start(xT[:, k, cs], xT_v[:, k, cs])
        nc.sync.dma_start(wp[:], wp_d.ap().rearrange("(k p) n -> p k n", p=P))
        nc.sync.dma_start(sel8[:].rearrange("p r m -> p (r m)"), sel_d.ap())
        nc.vector.memset(vsb[:, :, :, 64:65], 1.0)

        # ---- Phase 1: QKV projections ----
        for _rep in range(reps):
            _phase12(nc, ps, py, pp, etp, ysp, zgp, z0p, zbp, ytp, op,
                     xT, wq, wk, wv, wp, mk, qT, kT, vsb, yT, sel8, out_d)

    nc.compile()
    return nc


def _phase12(nc, ps, py, pp, etp, ysp, zgp, z0p, zbp, ytp, op,
             xT, wq, wk, wv, wp, mk, qT, kT, vsb, yT, sel8, out_d):
        def qkv_chunk(ch):
            cs = slice(ch * 512, (ch + 1) * 512)
            for p in range(NPAIR):
                psq = py.tile([P, 512], F32, name="psq", tag="psy")
                for k in range(CK):
                    nc.tensor.matmul(
                        psq[:], wq[:, k, p * P:(p + 1) * P], xT[:, k, cs],
                        start=(k == 0), stop=(k == CK - 1))
                nc.vector.tensor_copy(qT[:, p, cs], psq[:])
                psk = py.tile([P, 512], F32, name="psk", tag="psy")
                for k in range(CK):
                    nc.tensor.matmul(
                        psk[:], wk[:, k, p * P:(p + 1) * P], xT[:, k, cs],
                        start=(k == 0), stop=(k == CK - 1))
                nc.vector.tensor_copy(kT[:, p, cs], psk[:])
            for sub in range(4):
                jt = ch * 4 + sub
                psv = py.tile([P, 512], F32, name="psv", tag="psy")
                for k in range(CK):
                    nc.tensor.matmul(
                        psv[:], xT[:, k, jt * P:(jt + 1) * P], wv[:, k, :],
                        start=(k == 0), stop=(k == CK - 1))
                nc.vector.tensor_copy(
                    vsb[:, :, jt, 0:64],
                    psv[:].rearrange("p (h d) -> p h d", h=NH))

        # ---- Phase 2a: scores/exp/EV for one query i-range.  Leaves the
        # unnormalized y halves in SBUF (bf16) and the 8 softmax
        # denominators gathered into one [8, 512] tile; normalization +
        # projection are deferred into the next range so their latency
        # hides behind its matmuls.
        def attn_core(r):
            njt = 4 * (r + 1)
            ysbs = []
            zg = zgp.tile([8, 512], F32)
            for p in range(NPAIR):
                psy = [py.tile([P, 512], F32, name=f"psy{hh}", tag="psy")
                       for hh in range(2)]
                prev = None

                def emit_ev(jt, et, last):
                    # diagonal j-tiles only contribute to columns >= nst
                    mj = jt - 4 * r
                    nst = P * mj if mj > 0 else 0
                    for hh in range(2):
                        nc.tensor.matmul(
                            psy[hh][0:65, nst:512],
                            vsb[:, 2 * p + hh, jt, :],
                            et[:, hh * 512 + nst:(hh + 1) * 512],
                            start=(jt == 0), stop=last)

                for jt in range(njt):
                    m = jt - 4 * r
                    nst = P * m if m >= 0 else 0  # causal-narrowed col start
                    pss = ps.tile([P, 1024], F32, name="pss", tag="ps")
                    for hh in range(2):
                        hs = slice(hh * 64, (hh + 1) * 64)
                        nc.tensor.matmul(
                            pss[:, hh * 512 + nst:(hh + 1) * 512],
                            kT[hs, p, jt * P:(jt + 1) * P],
                            qT[hs, p, r * 512 + nst:(r + 1) * 512],
                            start=True, stop=True)
                    et = etp.tile([P, 1024], BF16)
                    scl = float(1.0 / np.sqrt(C))
                    if m < 0:
                        nc.scalar.activation(
                            et[:], pss[:], mybir.ActivationFunctionType.Exp,
                            scale=scl)
                    else:
                        ev3 = et[:].rearrange("q (t n) -> q t n", t=2)
                        pv3 = pss[:].rearrange("q (t n) -> q t n", t=2)
                        nc.scalar.activation(
                            ev3[:, :, nst:], pv3[:, :, nst:],
                            mybir.ActivationFunctionType.Exp, scale=scl)
                        # only the 128-wide diagonal band needs masking
                        tri = mk[:, m * 512 + nst:m * 512 + nst + P]
                        for hh in range(2):
                            nc.vector.tensor_mul(
                                et[:, hh * 512 + nst:hh * 512 + nst + P],
                                et[:, hh * 512 + nst:hh * 512 + nst + P],
                                tri)
                    if prev is not None:
                        emit_ev(jt - 1, prev, last=False)
                    prev = et
                emit_ev(njt - 1, prev, last=True)

                # evacuate psy immediately so the banks recycle to the
                # next pair's EV matmuls: y halves to SBUF bf16 (vector /
                # scalar), Z rows via 1-descriptor DMAs into the shared
                # [8, 512] gather tile.
                pair_ysb = []
                for hh in range(2):
                    ys = ysp.tile([64, 512], BF16)
                    zr = z0p.tile([1, 512], F32)
                    if hh == 0:
                        nc.vector.tensor_copy(ys[:], psy[hh][0:64, :])
                        nc.vector.tensor_copy(zr[:], psy[hh][64:65, :])
                    else:
                        nc.scalar.copy(ys[:], psy[hh][0:64, :])
                        nc.scalar.copy(zr[:], psy[hh][64:65, :])
                    row = 2 * p + hh
                    nc.sync.dma_start(zg[row:row + 1, :], zr[:])
                    pair_ysb.append(ys)
                ysbs.append(pair_ysb)
            return ysbs, zg

        # ---- Phase 2b: softmax normalization + output projection for a
        # completed range.  One [8, 512] reciprocal covers all 8 head
        # pairs (DVE time scales with free size only, so batching rows is
        # 8x cheaper than 8 separate [1, 512] reciprocals).
        def norm_and_proj(r, ysbs, zg, final=False):
            rs = slice(r * 512, (r + 1) * 512)
            rz8 = zgp.tile([8, 512], F32)
            nc.vector.reciprocal(rz8[:], zg[:])
            if final:
                rz8b = zgp.tile([8, 512], BF16)
                nc.vector.tensor_copy(rz8b[:], rz8[:])
            for p in range(NPAIR):
                for hh in range(2):
                    row = 2 * p + hh
                    if final:
                        # exposed tail: broadcast on the (otherwise idle)
                        # PE -- one-hot selector column picks row `row`
                        # of rz8b and replicates it onto 64 partitions.
                        # No partition-0 DMA hop, no serial 1.4us gpsimd
                        # broadcasts.
                        zb = py.tile([64, 512], F32, name="zbp", tag="psy")
                        nc.tensor.matmul(zb[:], sel8[:, row, :], rz8b[:],
                                         start=True, stop=True)
                    else:
                        r0 = z0p.tile([1, 512], F32)
                        nc.sync.dma_start(r0[:], rz8[row:row + 1, :])
                        zb = zbp.tile([64, 512], F32)
                        nc.gpsimd.partition_broadcast(zb[:], r0[:])
                    if hh == 0:
                        nc.vector.tensor_mul(
                            yT[0:64, p, rs], ysbs[p][hh][:], zb[:])
                    else:
                        yt = ytp.tile([64, 512], BF16)
                        nc.vector.tensor_mul(yt[:], ysbs[p][hh][:], zb[:])
                        for c4 in range(4):
                            cs = slice(c4 * 128, (c4 + 1) * 128)
                            nc.sync.dma_start(
                                yT[64:128, p, r * 512 + c4 * 128:
                                   r * 512 + (c4 + 1) * 128],
                                yt[:, cs])

            for it in range(4):
                tok = r * 512 + it * P
                obuf = op.tile([P, C], F32)
                pph = [pp.tile([P, 512], F32, name=f"pph{nh}", tag="pph")
                       for nh in range(2)]
                for p in range(NPAIR):
                    for nh in range(2):
                        nc.tensor.matmul(
                            pph[nh][:], yT[:, p, tok:tok + P],
                            wp[:, p, nh * 512:(nh + 1) * 512],
                            start=(p == 0), stop=(p == NPAIR - 1))
                nc.scalar.copy(obuf[:, 0:512], pph[0][:])
                nc.vector.tensor_copy(obuf[:, 512:1024], pph[1][:])
                nc.sync.dma_start(out_d.ap()[tok:tok + P, :], obuf[:])

        # interleave: attention for range r only needs QKV chunks 0..r,
        # and range r's normalization + projection are issued after range
        # r+1's scores/EV so every serial epilogue chain (reciprocal,
        # broadcast, yT writes) executes under the next range's matmuls.
        # Only the last range's epilogue is exposed at the end.
        state = {}
        for ch in range(NCH):
            qkv_chunk(ch)
            state[ch] = attn_core(ch)
            if ch > 0:
                norm_and_proj(ch - 1, *state.pop(ch - 1))
        norm_and_proj(NCH - 1, *state.pop(NCH - 1), final=True)


def get_program(reps=1):
    global _COMPILED
    if _COMPILED is None:
        _COMPILED = _build_program(reps=reps)
    return _COMPILED


def make_in_maps(x, W_attn, W_proj):
    bf = ml_dtypes.bfloat16
    x = np.asarray(x, np.float32)
    W_attn = np.asarray(W_attn, np.float32)
    W_proj = np.asarray(W_proj, np.float32)

    # causal sub-tile masks for the 4 diagonal positions of a 512-wide
    # i-range: mask[m][j, i_local] = (i_local >= 128*m + j)
    i_loc = np.arange(512)[None, :]
    j_loc = np.arange(P)[:, None]
    mk = np.concatenate(
        [(i_loc >= P * m + j_loc) for m in range(4)], axis=1).astype(bf)

    # one-hot selector columns: sel[k, row*64 + m] = (k == row), used to
    # broadcast row `row` of an [8, 512] tile across 64 PSUM partitions
    # via a K=8 matmul
    sel = np.zeros((8, 8, 64), np.float32)
    for k in range(8):
        sel[k, k, :] = 1.0
    sel = sel.reshape(8, 512).astype(bf)

    in_maps = []
    for c in range(NCORES):
        b, hg = c // 2, c % 2
        cols = slice(hg * 512, hg * 512 + 512)
        in_maps.append({
            "xt": np.ascontiguousarray(x[b].T.astype(bf)),
            "wq": np.ascontiguousarray(W_attn[:, cols].astype(bf)),
            "wk": np.ascontiguousarray(W_attn[:, 1024:2048][:, cols].astype(bf)),
            "wv": np.ascontiguousarray(W_attn[:, 2048:3072][:, cols].astype(bf)),
            "wp": np.ascontiguousarray(W_proj[hg * 512:hg * 512 + 512, :].astype(bf)),
            "mk": mk,
            "sel": sel,
        })
    return in_maps


def combine_outputs(results):
    out = np.zeros((B, L, C), np.float32)
    for c in range(NCORES):
        out[c // 2] += results[c]["out"]
    return out


def kernel(x, W_attn, W_proj):
    nc = get_program()
    in_maps = make_in_maps(x, W_attn, W_proj)
    res = run_bass_kernel_spmd(nc, in_maps, list(range(NCORES)))
    return combine_outputs(res.results)



# revision 32
# speedup vs baseline: 1.0168x; 1.0168x over previous
"""Causal multi-head attention block on 8 trn2 NeuronCores.

Problem (hardcoded): x [4, 2048, 1024] fp32, W_attn [1024, 3072], W_proj
[1024, 1024]; H=16 heads, D=64; scores scaled by 1/sqrt(1024); causal
softmax; y @ W_proj.

Sharding: core c -> (batch b = c//2, head-group hg = c%2 of 8 heads).
Each core computes q,k,v for its batch + head-group, causal attention,
and a partial projection out_partial = y_slice @ W_proj[rows of its
head-group].  Host sums the two partials per batch.

Device-side layout trick: everything is computed transposed (d on
partitions, tokens on the free axis) so no on-device transposes are
needed:
  qT/kT = W_slice.T @ xT          (xT passed pre-transposed from host)
  sT[j,i] = k_j . q_i             (lhsT = kT tile, rhs = qT range)
  ET = exp(sT/32)                 (no max-subtraction: |s/32| < ~1.5 by
                                   construction of the input distribution)
  yT_un[d,i], Z[i] = v_aug.T @ ET (v_aug has an all-ones 65th column, so
                                   row 64 of the product is the softmax
                                   denominator -- free on the PE)
  out = (yT_un/Z).T @ W_proj_slice
"""

import os
from contextlib import ExitStack

import numpy as np
import ml_dtypes

import concourse.bass as bass
import concourse.mybir as mybir
from concourse import bacc, tile
from concourse.bass_utils import run_bass_kernel_spmd

B, L, C, H, D = 4, 2048, 1024, 16, 64
P = 128
NCORES = 8
NH = 8          # heads per core
NPAIR = 4       # head pairs per core
CK = C // P     # 8 contraction k-tiles over C
NCH = 4         # 512-token chunks per batch
NR = 4          # query i-ranges of 512
NJT = 16        # key j-tiles of 128
BF16 = mybir.dt.bfloat16
F32 = mybir.dt.float32

_COMPILED = None


def _build_program(reps=1):
    nc = bacc.Bacc("TRN2", target_bir_lowering=False, debug=False,
                   num_devices=NCORES)
    xT_d = nc.dram_tensor("xt", [C, L], BF16, kind="ExternalInput")
    wq_d = nc.dram_tensor("wq", [C, 512], BF16, kind="ExternalInput")
    wk_d = nc.dram_tensor("wk", [C, 512], BF16, kind="ExternalInput")
    wv_d = nc.dram_tensor("wv", [C, 512], BF16, kind="ExternalInput")
    wp_d = nc.dram_tensor("wp", [512, C], BF16, kind="ExternalInput")
    mk_d = nc.dram_tensor("mk", [P, 2048], BF16, kind="ExternalInput")
    sel_d = nc.dram_tensor("sel", [8, 512], BF16, kind="ExternalInput")
    out_d = nc.dram_tensor("out", [L, C], F32, kind="ExternalOutput")

    with tile.TileContext(nc) as tc, ExitStack() as ctx:
        const = ctx.enter_context(tc.tile_pool(name="const", bufs=1))
        etp = ctx.enter_context(tc.tile_pool(name="et", bufs=4))
        ysp = ctx.enter_context(tc.tile_pool(name="ys", bufs=16))
        zgp = ctx.enter_context(tc.tile_pool(name="zg", bufs=3))
        z0p = ctx.enter_context(tc.tile_pool(name="z0", bufs=3))
        zbp = ctx.enter_context(tc.tile_pool(name="zb", bufs=2))
        ytp = ctx.enter_context(tc.tile_pool(name="ytmp", bufs=4))
        op = ctx.enter_context(tc.tile_pool(name="ob", bufs=2))
        ps = ctx.enter_context(
            tc.tile_pool(name="ps", bufs=2, space=bass.MemorySpace.PSUM))
        py = ctx.enter_context(
            tc.tile_pool(name="py", bufs=2, space=bass.MemorySpace.PSUM))
        pp = ctx.enter_context(
            tc.tile_pool(name="pp", bufs=2, space=bass.MemorySpace.PSUM))

        xT = const.tile([P, CK, L], BF16)
        wq = const.tile([P, CK, 512], BF16)
        wk = const.tile([P, CK, 512], BF16)
        wv = const.tile([P, CK, 512], BF16)
        wp = const.tile([P, NPAIR, C], BF16)
        mk = const.tile([P, 2048], BF16)
        qT = const.tile([P, NPAIR, L], BF16)
        kT = const.tile([P, NPAIR, L], BF16)
        vsb = const.tile([P, NH, NJT, 65], BF16)
        yT = const.tile([P, NPAIR, L], BF16)
        sel8 = const.tile([8, 8, 64], BF16)

        # DMA in consumption order: first QKV chunk needs wq + xT chunk 0
        # first, so the PE can start ~14us earlier than a bulk load.
        xT_v = xT_d.ap().rearrange("(k p) n -> p k n", p=P)
        nc.sync.dma_start(wq[:], wq_d.ap().rearrange("(k p) n -> p k n", p=P))
        for k in range(CK):
            nc.sync.dma_start(xT[:, k, 0:512], xT_v[:, k, 0:512])
        nc.sync.dma_start(wk[:], wk_d.ap().rearrange("(k p) n -> p k n", p=P))
        nc.sync.dma_start(wv[:], wv_d.ap().rearrange("(k p) n -> p k n", p=P))
        nc.sync.dma_start(mk[:], mk_d.ap())
        for ch in range(1, NCH):
            cs = slice(ch * 512, (ch + 1) * 512)
            for k in range(CK):
                nc.sync.dma_start(xT[:, k, cs], xT_v[:, k, cs])
        nc.sync.dma_start(wp[:], wp_d.ap().rearrange("(k p) n -> p k n", p=P))
        nc.sync.dma_start(sel8[:].rearrange("p r m -> p (r m)"), sel_d.ap())
        nc.vector.memset(vsb[:, :, :, 64:65], 1.0)

        # ---- Phase 1: QKV projections ----
        for _rep in range(reps):
            _phase12(nc, ps, py, pp, etp, ysp, zgp, z0p, zbp, ytp, op,
                     xT, wq, wk, wv, wp, mk, qT, kT, vsb, yT, sel8, out_d)

    nc.compile()
    return nc


def _phase12(nc, ps, py, pp, etp, ysp, zgp, z0p, zbp, ytp, op,
             xT, wq, wk, wv, wp, mk, qT, kT, vsb, yT, sel8, out_d):
        def qkv_chunk(ch):
            cs = slice(ch * 512, (ch + 1) * 512)
            for p in range(NPAIR):
                psq = ps.tile([P, 512], F32, name="psq", tag="ps")
                for k in range(CK):
                    nc.tensor.matmul(
                        psq[:], wq[:, k, p * P:(p + 1) * P], xT[:, k, cs],
                        start=(k == 0), stop=(k == CK - 1))
                nc.vector.tensor_copy(qT[:, p, cs], psq[:])
                psk = ps.tile([P, 512], F32, name="psk", tag="ps")
                for k in range(CK):
                    nc.tensor.matmul(
                        psk[:], wk[:, k, p * P:(p + 1) * P], xT[:, k, cs],
                        start=(k == 0), stop=(k == CK - 1))
                nc.vector.tensor_copy(kT[:, p, cs], psk[:])
            for sub in range(4):
                jt = ch * 4 + sub
                psv = ps.tile([P, 512], F32, name="psv", tag="ps")
                for k in range(CK):
                    nc.tensor.matmul(
                        psv[:], xT[:, k, jt * P:(jt + 1) * P], wv[:, k, :],
                        start=(k == 0), stop=(k == CK - 1))
                nc.vector.tensor_copy(
                    vsb[:, :, jt, 0:64],
                    psv[:].rearrange("p (h d) -> p h d", h=NH))

        # ---- Phase 2a: scores/exp/EV for one query i-range.  Leaves the
        # unnormalized y halves in SBUF (bf16) and the 8 softmax
        # denominators gathered into one [8, 512] tile; normalization +
        # projection are deferred into the next range so their latency
        # hides behind its matmuls.
        def attn_core(r):
            njt = 4 * (r + 1)
            ysbs = []
            zg = zgp.tile([8, 512], F32)
            for p in range(NPAIR):
                psy = [py.tile([P, 512], F32, name=f"psy{hh}", tag="psy")
                       for hh in range(2)]
                prev = None

                def emit_ev(jt, et, last):
                    # diagonal j-tiles only contribute to columns >= nst
                    mj = jt - 4 * r
                    nst = P * mj if mj > 0 else 0
                    for hh in range(2):
                        nc.tensor.matmul(
                            psy[hh][0:65, nst:512],
                            vsb[:, 2 * p + hh, jt, :],
                            et[:, hh * 512 + nst:(hh + 1) * 512],
                            start=(jt == 0), stop=last)

                for jt in range(njt):
                    m = jt - 4 * r
                    nst = P * m if m >= 0 else 0  # causal-narrowed col start
                    pss = ps.tile([P, 1024], F32, name="pss", tag="ps")
                    for hh in range(2):
                        hs = slice(hh * 64, (hh + 1) * 64)
                        nc.tensor.matmul(
                            pss[:, hh * 512 + nst:(hh + 1) * 512],
                            kT[hs, p, jt * P:(jt + 1) * P],
                            qT[hs, p, r * 512 + nst:(r + 1) * 512],
                            start=True, stop=True)
                    et = etp.tile([P, 1024], BF16)
                    scl = float(1.0 / np.sqrt(C))
                    if m < 0:
                        nc.scalar.activation(
                            et[:], pss[:], mybir.ActivationFunctionType.Exp,
                            scale=scl)
                    else:
                        ev3 = et[:].rearrange("q (t n) -> q t n", t=2)
                        pv3 = pss[:].rearrange("q (t n) -> q t n", t=2)
                        nc.scalar.activation(
                            ev3[:, :, nst:], pv3[:, :, nst:],
                            mybir.ActivationFunctionType.Exp, scale=scl)
                        # only the 128-wide diagonal band needs masking
                        tri = mk[:, m * 512 + nst:m * 512 + nst + P]
                        for hh in range(2):
                            nc.vector.tensor_mul(
                                et[:, hh * 512 + nst:hh * 512 + nst + P],
                                et[:, hh * 512 + nst:hh * 512 + nst + P],
                                tri)
                    if prev is not None:
                        emit_ev(jt - 1, prev, last=False)
                    prev = et
                emit_ev(njt - 1, prev, last=True)

                # evacuate psy immediately so the banks recycle to the
                # next pair's EV matmuls: y halves to SBUF bf16 (vector /
                # scalar), Z rows via 1-descriptor DMAs into the shared
                # [8, 512] gather tile.
                pair_ysb = []
                for hh in range(2):
                    ys = ysp.tile([64, 512], BF16)
                    zr = z0p.tile([1, 512], F32)
                    if hh == 0:
                        nc.vector.tensor_copy(ys[:], psy[hh][0:64, :])
                        nc.vector.tensor_copy(zr[:], psy[hh][64:65, :])
                    else:
                        nc.scalar.copy(ys[:], psy[hh][0:64, :])
                        nc.scalar.copy(zr[:], psy[hh][64:65, :])
                    row = 2 * p + hh
                    nc.sync.dma_start(zg[row:row + 1, :], zr[:])
                    pair_ysb.append(ys)
                ysbs.append(pair_ysb)
            return ysbs, zg

        # ---- Phase 2b: softmax normalization + output projection for a
        # completed range.  One [8, 512] reciprocal covers all 8 head
        # pairs (DVE time scales with free size only, so batching rows is
        # 8x cheaper than 8 separate [1, 512] reciprocals).
        def norm_and_proj(r, ysbs, zg, final=False):
            rs = slice(r * 512, (r + 1) * 512)
            rz8 = zgp.tile([8, 512], F32)
            nc.vector.reciprocal(rz8[:], zg[:])
            if final:
                rz8b = zgp.tile([8, 512], BF16)
                nc.vector.tensor_copy(rz8b[:], rz8[:])
            for p in range(NPAIR):
                for hh in range(2):
                    row = 2 * p + hh
                    if final:
                        # exposed tail: broadcast on the (otherwise idle)
                        # PE -- one-hot selector column picks row `row`
                        # of rz8b and replicates it onto 64 partitions.
                        # No partition-0 DMA hop, no serial 1.4us gpsimd
                        # broadcasts.
                        zb = py.tile([64, 512], F32, name="zbp", tag="psy")
                        nc.tensor.matmul(zb[:], sel8[:, row, :], rz8b[:],
                                         start=True, stop=True)
                    else:
                        r0 = z0p.tile([1, 512], F32)
                        nc.sync.dma_start(r0[:], rz8[row:row + 1, :])
                        zb = zbp.tile([64, 512], F32)
                        nc.gpsimd.partition_broadcast(zb[:], r0[:])
                    if hh == 0:
                        nc.vector.tensor_mul(
                            yT[0:64, p, rs], ysbs[p][hh][:], zb[:])
                    else:
                        yt = ytp.tile([64, 512], BF16)
                        nc.vector.tensor_mul(yt[:], ysbs[p][hh][:], zb[:])
                        for c4 in range(4):
                            cs = slice(c4 * 128, (c4 + 1) * 128)
                            nc.sync.dma_start(
                                yT[64:128, p, r * 512 + c4 * 128:
                                   r * 512 + (c4 + 1) * 128],
                                yt[:, cs])

            for it in range(4):
                tok = r * 512 + it * P
                obuf = op.tile([P, C], F32)
                pph = [pp.tile([P, 512], F32, name=f"pph{nh}", tag="pph")
                       for nh in range(2)]
                for p in range(NPAIR):
                    for nh in range(2):
                        nc.tensor.matmul(
                            pph[nh][:], yT[:, p, tok:tok + P],
                            wp[:, p, nh * 512:(nh + 1) * 512],
                            start=(p == 0), stop=(p == NPAIR - 1))
                nc.scalar.copy(obuf[:, 0:512], pph[0][:])
                nc.vector.tensor_copy(obuf[:, 512:1024], pph[1][:])
                nc.sync.dma_start(out_d.ap()[tok:tok + P, :], obuf[:])

        # interleave: attention for range r only needs QKV chunks 0..r,
        # and range r's normalization + projection are issued after range
        # r+1's scores/EV so every serial epilogue chain (reciprocal,
        # broadcast, yT writes) executes under the next range's matmuls.
        # Only the last range's epilogue is exposed at the end.
        state = {}
        for ch in range(NCH):
            qkv_chunk(ch)
            state[ch] = attn_core(ch)
            if ch > 0:
                norm_and_proj(ch - 1, *state.pop(ch - 1))
        norm_and_proj(NCH - 1, *state.pop(NCH - 1), final=True)


def get_program(reps=1):
    global _COMPILED
    if _COMPILED is None:
        _COMPILED = _build_program(reps=reps)
    return _COMPILED


def make_in_maps(x, W_attn, W_proj):
    bf = ml_dtypes.bfloat16
    x = np.asarray(x, np.float32)
    W_attn = np.asarray(W_attn, np.float32)
    W_proj = np.asarray(W_proj, np.float32)

    # causal sub-tile masks for the 4 diagonal positions of a 512-wide
    # i-range: mask[m][j, i_local] = (i_local >= 128*m + j)
    i_loc = np.arange(512)[None, :]
    j_loc = np.arange(P)[:, None]
    mk = np.concatenate(
        [(i_loc >= P * m + j_loc) for m in range(4)], axis=1).astype(bf)

    # one-hot selector columns: sel[k, row*64 + m] = (k == row), used to
    # broadcast row `row` of an [8, 512] tile across 64 PSUM partitions
    # via a K=8 matmul
    sel = np.zeros((8, 8, 64), np.float32)
    for k in range(8):
        sel[k, k, :] = 1.0
    sel = sel.reshape(8, 512).astype(bf)

    in_maps = []
    for c in range(NCORES):
        b, hg = c // 2, c % 2
        cols = slice(hg * 512, hg * 512 + 512)
        in_maps.append({
            "xt": np.ascontiguousarray(x[b].T.astype(bf)),
            "wq": np.ascontiguousarray(W_attn[:, cols].astype(bf)),
            "wk": np.ascontiguousarray(W_attn[:, 1024:2048][:, cols].astype(bf)),
            "wv": np.ascontiguousarray(W_attn[:, 2048:3072][:, cols].astype(bf)),
            "wp": np.ascontiguousarray(W_proj[hg * 512:hg * 512 + 512, :].astype(bf)),
            "mk": mk,
            "sel": sel,
        })
    return in_maps


def combine_outputs(results):
    out = np.zeros((B, L, C), np.float32)
    for c in range(NCORES):
        out[c // 2] += results[c]["out"]
    return out


def kernel(x, W_attn, W_proj):
    nc = get_program()
    in_maps = make_in_maps(x, W_attn, W_proj)
    res = run_bass_kernel_spmd(nc, in_maps, list(range(NCORES)))
    return combine_outputs(res.results)

